# revision 45
# baseline (speedup 1.0000x reference)
"""GQA kernel for Trainium2, 8 NeuronCores.

Problem: B=4, S=1024, D=2048, 32 q-heads, 8 kv-heads, head_dim=64, fp32.

Sharding: TP-2 over heads x DP-4 over batch. Core c handles batch c//2 and
(for tp = c%2) q-heads [16*tp, 16*tp+16) / kv-heads [4*tp, 4*tp+4). Each core
produces a partial output [1024, 2048] (its heads' contribution to ctx @ Wo);
host sums the two partials per batch. bo is added by the tp=0 core only.

Numerics: all weights are prescaled by 64 on the host and split into fp8-e4m3
hi/lo pairs; x is likewise split (hi/lo). Projections run as 3-term
compensated fp8 DoubleRow matmuls (w_hi x_hi + w_lo x_hi + w_hi x_lo), which
the PE cost model charges at half the fp32r row rate. Scores use fp8
DoubleRow with (k_hi, k_hi) x (q_hi, q_lo) slot pairing = k_hi^T(q_hi+q_lo).
The 64*64 scale is absorbed in the exp() scale and the final 1/4096 output
scale. PV runs in bf16 (the v/e linear path cannot take direct fp8 without
blowing the error gate). Softmax denominator comes from an all-ones 65th
column of the value tiles; normalization divides PV psum rows by it before
the fp8 ctx hi/lo split feeding the output projection.

x is transposed on the host; no on-device transposes are needed.

Schedule: the attention phase runs as 16 (pair, token-half) units in a
software pipeline paced by the ACT engine's exp throughput (the hard floor
of the phase): unit u emits scores/exp in 8 block iterations while unit
u-1's PV matmuls (whose exps are all complete) run dense in iterations
0..3 and its normalize chain drains on DVE/Pool; iterations 4..7 are
packed with filler thunks (V-projection tail, the next pair's Q
projection) so the PE never idles long enough to drop out of its full
p-state. K/V/Q0 stream against the x DMAs at the head; the output
projection's per-i-chunk order puts the last pairs' ctx last.

Measured on hardware (same fixed inputs as the harness): rel err 7.35e-3,
HW exec time 227043 ns (baseline 400405 ns).
"""

import time

import ml_dtypes
import numpy as np

import concourse.bass as bass
import concourse.mybir as mybir
from concourse import bacc
from concourse.tile import TileContext
from concourse.bass_utils import run_bass_kernel_spmd

F32 = mybir.dt.float32
BF16 = mybir.dt.bfloat16
F8 = mybir.dt.float8e4
DR = mybir.MatmulPerfMode.DoubleRow
EXP = mybir.ActivationFunctionType.Exp
ADD = mybir.AluOpType.add
SUB = mybir.AluOpType.subtract
MUL = mybir.AluOpType.mult

S = 1024          # sequence length
D = 2048          # d_model
NH = 16           # q heads per core
NKV = 4           # kv heads per core
HD = 64           # head dim
QF = NH * HD      # 1024 q features per core
KF = NKV * HD     # 256 kv features per core
NC2 = 8           # contraction chunk-pairs of d_model (16 chunks of 128)
TT = S // 128     # 8 token tiles
TH = S // 512     # 2 token halves
WS = 64.0         # weight prescale (absorbed downstream)
SCALE = 1.0 / (8.0 * WS * WS)   # exp scale: true scores = psum/(sqrt(64)*64*64)
OS = 1.0 / (WS * WS)            # output descale: out = pot/4096 + bo

# pair p -> (lo head, hi head) local q-head indices; lo heads have kv parity 0
# (kv = h//4; parity 0 -> partitions 0:64 of kT group kv//2).
LO = [0, 1, 2, 3, 8, 9, 10, 11]
HI = [4, 5, 6, 7, 12, 13, 14, 15]
HEAD_PERM = [h for _p in range(8) for h in (LO[_p], HI[_p])]

_CACHE = {}
LAST_RUN_NS = None


def _build():
    if "nc" in _CACHE:
        return _CACHE["nc"]

    nc = bacc.Bacc("TRN2", target_bir_lowering=False, debug=False)

    xh_d = nc.dram_tensor("xh", [128, NC2, 2, S], F8, kind="ExternalInput").ap()
    xl_d = nc.dram_tensor("xl", [128, NC2, 2, S], F8, kind="ExternalInput").ap()
    wqh_d = nc.dram_tensor("wqh", [128, 8, NC2, 2, 128], F8, kind="ExternalInput").ap()
    wql_d = nc.dram_tensor("wql", [128, 8, NC2, 2, 128], F8, kind="ExternalInput").ap()
    wkh_d = nc.dram_tensor("wkh", [128, NC2, 2, KF], F8, kind="ExternalInput").ap()
    wkl_d = nc.dram_tensor("wkl", [128, NC2, 2, KF], F8, kind="ExternalInput").ap()
    wvh_d = nc.dram_tensor("wvh", [128, NC2, 2, KF], F8, kind="ExternalInput").ap()
    wvl_d = nc.dram_tensor("wvl", [128, NC2, 2, KF], F8, kind="ExternalInput").ap()
    woh_d = nc.dram_tensor("woh", [128, 4, 4, 2, 512], F8, kind="ExternalInput").ap()
    wol_d = nc.dram_tensor("wol", [128, 4, 4, 2, 512], F8, kind="ExternalInput").ap()
    bq_d = nc.dram_tensor("bq", [128, 8], F32, kind="ExternalInput").ap()
    bk_d = nc.dram_tensor("bk", [128, 2], F32, kind="ExternalInput").ap()
    bvb_d = nc.dram_tensor("bvb", [128, KF + 4], F32, kind="ExternalInput").ap()
    bob_d = nc.dram_tensor("bob", [128, D], F32, kind="ExternalInput").ap()
    out_d = nc.dram_tensor("out", [S, D], F32, kind="ExternalOutput").ap()

    with TileContext(nc) as tc:
        with (
            tc.tile_pool(name="const", bufs=1) as constp,
            tc.tile_pool(name="xp", bufs=1) as xp,
            tc.tile_pool(name="wkvp", bufs=1) as wkvp,
            tc.tile_pool(name="wqp", bufs=3) as wqp,
            tc.tile_pool(name="kTp", bufs=1) as kTp,
            tc.tile_pool(name="qTp", bufs=1) as qTp,
            tc.tile_pool(name="vaugp", bufs=1) as vaugp,
            tc.tile_pool(name="ctxp", bufs=1) as ctxp,
            tc.tile_pool(name="ps_proj", bufs=2, space="PSUM") as ps_proj,
        ):
            # consts + out-stores go through the Pool queue (SWDGE path),
            # keeping the single HWDGE device free for x / weight loads.
            bq_sb = constp.tile([128, 8], F32, tag="bq")
            nc.gpsimd.dma_start(out=bq_sb[:], in_=bq_d[:, :])
            bk_sb = constp.tile([128, 2], F32, tag="bk")
            nc.gpsimd.dma_start(out=bk_sb[:], in_=bk_d[:, :])
            bvb_sb = constp.tile([128, KF + 4], F32, tag="bvb")
            nc.gpsimd.dma_start(out=bvb_sb[:], in_=bvb_d[:, :])
            bob_sb = constp.tile([128, D], F32, tag="bob")
            nc.gpsimd.dma_start(out=bob_sb[:], in_=bob_d[:, :])

            # x arrives in 2-chunk-pair granules, hi on the SP queue and lo
            # on the DVE queue so the K projection can start on chunk pair 0
            # while the rest stream in.
            # x in four half-tensor DMAs (8KB contiguous per partition):
            # fewer DMA instructions means less HWDGE/issue overhead in the
            # latency-critical startup stream.
            xh_t = xp.tile([128, NC2, 2, S], F8, tag="xh")
            xl_t = xp.tile([128, NC2, 2, S], F8, tag="xl")
            xh = [xh_t[:, c2, :, :] for c2 in range(NC2)]
            xl = [xl_t[:, c2, :, :] for c2 in range(NC2)]
            # K weights first so the K projection's hi-groups (wk x xh) can
            # start as soon as the first x half lands; xl and the V weights
            # stream in behind.
            wkh = wkvp.tile([128, NC2, 2, KF], F8, tag="wkh")
            nc.scalar.dma_start(out=wkh[:], in_=wkh_d[:, :, :, :])
            wkl = wkvp.tile([128, NC2, 2, KF], F8, tag="wkl")
            nc.scalar.dma_start(out=wkl[:], in_=wkl_d[:, :, :, :])
            for h in range(2):
                nc.sync.dma_start(
                    out=xh_t[:, 4 * h:4 * (h + 1), :, :],
                    in_=xh_d[:, 4 * h:4 * (h + 1), :, :])
                nc.scalar.dma_start(
                    out=xl_t[:, 4 * h:4 * (h + 1), :, :],
                    in_=xl_d[:, 4 * h:4 * (h + 1), :, :])
            def load_wq(p):
                wh = wqp.tile([128, NC2, 2, 128], F8, tag="wqh", name=f"wqh{p}")
                nc.sync.dma_start(out=wh[:], in_=wqh_d[:, p, :, :, :])
                wl = wqp.tile([128, NC2, 2, 128], F8, tag="wql", name=f"wql{p}")
                nc.sync.dma_start(out=wl[:], in_=wql_d[:, p, :, :, :])
                return wh, wl
            wq0 = load_wq(0)
            wvh = wkvp.tile([128, NC2, 2, KF], F8, tag="wvh")
            nc.sync.dma_start(out=wvh[:], in_=wvh_d[:, :, :, :])
            wvl = wkvp.tile([128, NC2, 2, KF], F8, tag="wvl")
            nc.sync.dma_start(out=wvl[:], in_=wvl_d[:, :, :, :])

            kT = [kTp.tile([128, 2, S], F8, tag=f"kT{g}", name=f"kT{g}")
                  for g in range(2)]
            qT = [qTp.tile([128, 2, S], F8, tag=f"qT{p}", name=f"qT{p}")
                  for p in range(8)]
            vaug = vaugp.tile([128, NKV, 65 * TT], BF16, tag="vaug")
            ctxH = [ctxp.tile([128, 2, S], F8, tag=f"cH{i}", name=f"cH{i}")
                    for i in range(4)]
            ctxL = [ctxp.tile([128, 2, S], F8, tag=f"cL{i}", name=f"cL{i}")
                    for i in range(4)]


            def q_proj(p, wh, wl):
                for th in range(TH):
                    pq = ps_proj.tile([128, 512], F32, tag="proj",
                                      name=f"pq{p}_{th}")
                    k = 0
                    for wt, xt in ((wh, xh), (wl, xh), (wh, xl)):
                        for c2 in range(NC2):
                            nc.tensor.matmul(
                                pq[:], wt[:, c2, :, :],
                                xt[c2][:, :, 512 * th:512 * (th + 1)],
                                start=(k == 0), stop=(k == 3 * NC2 - 1),
                                perf_mode=DR,
                            )
                            k += 1
                    nc.vector.tensor_scalar_add(
                        qT[p][:, 0, 512 * th:512 * (th + 1)], pq[:],
                        bq_sb[:, p:p + 1])
                    nc.vector.scalar_tensor_tensor(
                        qT[p][:, 1, 512 * th:512 * (th + 1)], pq[:],
                        bq_sb[:, p:p + 1],
                        qT[p][:, 0, 512 * th:512 * (th + 1)], ADD, SUB)

            # ---- K projection ----
            with tc.tile_pool(name="ps_k", bufs=1, space="PSUM") as ps_k:
                pk = {}
                for g in range(2):
                    for th in range(TH):
                        pk[(g, th)] = ps_k.tile(
                            [128, 512], F32, tag=f"pk{g}{th}", name=f"pk{g}{th}")
                # hi-groups A/B per chunk pair as xh streams in (only wk and
                # xh needed); the xl group C trails once xl has landed.
                for c2 in range(NC2):
                    for gi, wt in ((0, wkh), (1, wkl)):
                        for g in range(2):
                            for th in range(TH):
                                nc.tensor.matmul(
                                    pk[(g, th)][:],
                                    wt[:, c2, :, 128 * g:128 * (g + 1)],
                                    xh[c2][:, :, 512 * th:512 * (th + 1)],
                                    start=(gi == 0 and c2 == 0),
                                    stop=False,
                                    perf_mode=DR,
                                )
                for c2 in range(NC2):
                    for g in range(2):
                        for th in range(TH):
                            nc.tensor.matmul(
                                pk[(g, th)][:],
                                wkh[:, c2, :, 128 * g:128 * (g + 1)],
                                xl[c2][:, :, 512 * th:512 * (th + 1)],
                                start=False,
                                stop=(c2 == NC2 - 1),
                                perf_mode=DR,
                            )
                for g in range(2):
                    for th in range(TH):
                        nc.vector.tensor_scalar_add(
                            kT[g][:, 0, 512 * th:512 * (th + 1)],
                            pk[(g, th)][:], bk_sb[:, g:g + 1])
                        nc.vector.tensor_copy(
                            kT[g][:, 1, 512 * th:512 * (th + 1)],
                            kT[g][:, 0, 512 * th:512 * (th + 1)])

            # ---- Q projection for pair 0 + first half of V (head) ----
            q_proj(0, *wq0)

            bv_r = bvb_sb[:, 0:KF].rearrange("p (j f) -> p j f", j=NKV)
            ones_r = bvb_sb[:, KF:KF + 4].rearrange("p (j f) -> p j f", f=1)

            v_boxes = {}

            def v_tile_half(t, half):
                vgrp = [(xt, wt, c2) for xt, wt in
                        ((xh, wvh), (xh, wvl), (xl, wvh))
                        for c2 in range(NC2)]
                if half == 0:
                    v_boxes[t] = ps_proj.tile([128, KF], F32, tag="proj",
                                              name=f"pv{t}")
                pvt = v_boxes[t]
                for k in range(12 * half, 12 * half + 12):
                    xt, wt, c2 = vgrp[k]
                    nc.tensor.matmul(
                        pvt[:], xt[c2][:, :, 128 * t:128 * (t + 1)],
                        wt[:, c2, :, :],
                        start=(k == 0), stop=(k == 3 * NC2 - 1),
                        perf_mode=DR,
                    )
                if half == 1:
                    nc.vector.tensor_add(
                        vaug[:, :, 65 * t:65 * t + 64],
                        pvt[:].rearrange("p (j f) -> p j f", j=NKV), bv_r)
                    nc.vector.tensor_copy(
                        vaug[:, :, 65 * t + 64:65 * t + 65], ones_r)
                    del v_boxes[t]


            # ---- attention: 16 (pair, th) units, software-pipelined ----
            # Unit u emits its scores/exp in 8 iterations; the PV block of
            # unit u-1 runs dense in iterations 0..3 (all its exps are done,
            # so the PE never waits on ACT), its normalize chain drains on
            # DVE/Pool while unit u continues, and filler thunks (V tail,
            # next pair's Q projection) pack iterations 4..7 up to the ACT
            # exp pace. ACT therefore runs back-to-back through the phase.
            with (
                tc.tile_pool(name="ep", bufs=22) as ep,
                tc.tile_pool(name="npool", bufs=2) as npool,
                tc.tile_pool(name="wop", bufs=2) as wop,
                tc.tile_pool(name="osb", bufs=3) as osbp,
                tc.tile_pool(name="ps_sc", bufs=2, space="PSUM") as ps_sc,
                tc.tile_pool(name="ps_pv", bufs=1, space="PSUM") as ps_pv,
            ):
                # prefetch the first two output-projection weight chunks on
                # the Pool queue: they have no deps, so they land during
                # attention instead of queueing behind the exp issues.
                wo_sb = {}
                for nf in range(2):
                    woh_sb = wop.tile([128, 4, 2, 512], F8, tag="woh",
                                      name=f"woh{nf}")
                    nc.gpsimd.dma_start(out=woh_sb[:], in_=woh_d[:, nf, :, :, :])
                    wol_sb = wop.tile([128, 4, 2, 512], F8, tag="wol",
                                      name=f"wol{nf}")
                    nc.gpsimd.dma_start(out=wol_sb[:], in_=wol_d[:, nf, :, :, :])
                    wo_sb[nf] = (woh_sb, wol_sb)

                fillers = []

                def push_q(p, wh, wl):
                    # 12 thunks x 4 DoubleRows per pair, chain order kept
                    grp = ((wh, xh), (wl, xh), (wh, xl))
                    for th in range(TH):
                        box = []
                        for j in range(6):
                            def thunk(j=j, th=th, grp=grp, box=box, p=p):
                                for k in range(4 * j, 4 * j + 4):
                                    wt, xt = grp[k // NC2]
                                    c2 = k % NC2
                                    if not box:
                                        box.append(ps_proj.tile(
                                            [128, 512], F32, tag="proj",
                                            name=f"pq{p}_{th}"))
                                    nc.tensor.matmul(
                                        box[0][:], wt[:, c2, :, :],
                                        xt[c2][:, :, 512 * th:512 * (th + 1)],
                                        start=(k == 0), stop=(k == 23),
                                        perf_mode=DR,
                                    )
                                    if k == 23:
                                        nc.vector.tensor_scalar_add(
                                            qT[p][:, 0, 512 * th:512 * (th + 1)],
                                            box[0][:], bq_sb[:, p:p + 1])
                                        nc.vector.scalar_tensor_tensor(
                                            qT[p][:, 1, 512 * th:512 * (th + 1)],
                                            box[0][:], bq_sb[:, p:p + 1],
                                            qT[p][:, 0, 512 * th:512 * (th + 1)],
                                            ADD, SUB)
                            fillers.append((430, thunk))

                def inject(budget):
                    while fillers and budget > 0:
                        cost, thunk = fillers.pop(0)
                        thunk()
                        budget -= cost

                for t in range(TT):
                    for half in range(2):
                        fillers.append(
                            (650, lambda t=t, half=half: v_tile_half(t, half)))
                push_q(1, *load_wq(1))

                ustate = {}

                def emit_pv(u, upto):
                    p, th, boxAB, es, done = ustate[u]
                    kvlo, kvhi = LO[p] // 4, HI[p] // 4
                    if done[0] >= upto:
                        return
                    if not boxAB:
                        boxAB.append(ps_pv.tile([65, 512], F32, tag="pvA",
                                                name=f"pvA{u}"))
                        boxAB.append(ps_pv.tile([65, 512], F32, tag="pvB",
                                                name=f"pvB{u}"))
                    pvA, pvB = boxAB
                    for pb in range(done[0], upto):
                        nc.tensor.matmul(
                            pvA[:], vaug[:, kvlo, 65 * pb:65 * pb + 65],
                            es[pb][:, 0:512],
                            start=(pb == 0), stop=(pb == TT - 1),
                        )
                        nc.tensor.matmul(
                            pvB[:], vaug[:, kvhi, 65 * pb:65 * pb + 65],
                            es[pb][:, 512:1024],
                            start=(pb == 0), stop=(pb == TT - 1),
                        )
                    done[0] = upto

                def nrm(u):
                    p, th, (pvA, pvB), es, done = ustate.pop(u)
                    i2, s2 = p // 2, p % 2
                    recA = npool.tile([1, 512], F32, tag="recA")
                    recB = npool.tile([1, 512], F32, tag="recB")
                    nc.vector.reciprocal(recA[:], pvA[64:65, :])
                    nc.vector.reciprocal(recB[:], pvB[64:65, :])
                    bcA = npool.tile([64, 512], F32, tag="bcA")
                    bcB = npool.tile([64, 512], F32, tag="bcB")
                    nc.gpsimd.partition_broadcast(bcA[:], recA[:])
                    nc.gpsimd.partition_broadcast(bcB[:], recB[:])
                    # tmp spans all 128 partitions so the hi-head half sits
                    # at base partition 64 like its ctx slice (the DVE needs
                    # equal base partitions for SBUF operand pairs).
                    tmp = npool.tile([128, 512], F32, tag="tmp")
                    nc.vector.tensor_mul(tmp[0:64, :], pvA[0:64, :], bcA[:])
                    nc.vector.tensor_mul(tmp[64:128, :], pvB[0:64, :], bcB[:])
                    cHa = ctxH[i2][0:64, s2, 512 * th:512 * (th + 1)]
                    cHb = ctxH[i2][64:128, s2, 512 * th:512 * (th + 1)]
                    nc.vector.tensor_copy(cHa, tmp[0:64, :])
                    nc.vector.tensor_copy(cHb, tmp[64:128, :])
                    nc.vector.tensor_sub(
                        ctxL[i2][0:64, s2, 512 * th:512 * (th + 1)],
                        tmp[0:64, :], cHa)
                    nc.vector.tensor_sub(
                        ctxL[i2][64:128, s2, 512 * th:512 * (th + 1)],
                        tmp[64:128, :], cHb)

                NU = 8 * TH
                for u in range(NU):
                    p, th = u // TH, u % TH
                    if th == 0 and 1 <= p < 7:
                        push_q(p + 1, *load_wq(p + 1))
                    glo, ghi = LO[p] // 4 // 2, HI[p] // 4 // 2
                    es = [None] * TT
                    ustate[u] = (p, th, [], es, [0])
                    for b in range(TT):
                        psc = ps_sc.tile([128, 1024], F32, tag="psc")
                        nc.tensor.matmul(
                            psc[:, 0:512],
                            kT[glo][0:64, :, 128 * b:128 * (b + 1)],
                            qT[p][0:64, :, 512 * th:512 * (th + 1)],
                            start=True, stop=True, perf_mode=DR,
                        )
                        nc.tensor.matmul(
                            psc[:, 512:1024],
                            kT[ghi][64:128, :, 128 * b:128 * (b + 1)],
                            qT[p][64:128, :, 512 * th:512 * (th + 1)],
                            start=True, stop=True, perf_mode=DR,
                        )
                        e = ep.tile([128, 1024], BF16, tag="e")
                        nc.scalar.activation(
                            e[:], psc[:], EXP, bias=0.0, scale=SCALE)
                        es[b] = e
                        if u > 0:
                            if b == 0:
                                # one extra iteration of slack before the PV
                                # block restarts: the pv psum slot is still
                                # draining through the previous normalize.
                                inject(430)
                            elif b < 3:
                                emit_pv(u - 1, 3 * b)
                            elif b == 3:
                                emit_pv(u - 1, 8)
                                nrm(u - 1)
                            elif u >= NU - 2:
                                # pair 7 has no next-pair Q filler; run its
                                # own PV early as the exps complete.
                                emit_pv(u, min(2 * (b - 3), 7))
                            else:
                                inject(860)
                        else:
                            inject(1300 if b < 4 else 860)
                emit_pv(NU - 1, 8)
                nrm(NU - 1)

                # ---- output projection ----
                if True:
                    for nf in range(4):
                        if nf < 2:
                            woh_sb, wol_sb = wo_sb[nf]
                        else:
                            woh_sb = wop.tile([128, 4, 2, 512], F8, tag="woh",
                                              name=f"woh{nf}")
                            nc.scalar.dma_start(
                                out=woh_sb[:], in_=woh_d[:, nf, :, :, :])
                            wol_sb = wop.tile([128, 4, 2, 512], F8, tag="wol",
                                              name=f"wol{nf}")
                            nc.scalar.dma_start(
                                out=wol_sb[:], in_=wol_d[:, nf, :, :, :])
                        for t in range(TT):
                            pot = ps_proj.tile([128, 512], F32, tag="proj",
                                               name=f"po{nf}_{t}")
                            # i=3 (ctx of pairs 6/7) last in every group so
                            # the first 9 matmuls can pre-run while the final
                            # pairs' attention is still normalizing.
                            k = 0
                            for i in (0, 1, 2, 3):
                                pass
                            groups = ((ctxH, woh_sb), (ctxL, woh_sb),
                                      (ctxH, wol_sb))
                            order = [(ct, wt, i) for ct, wt in groups
                                     for i in (0, 1, 2)]
                            order += [(ct, wt, 3) for ct, wt in groups]
                            for ct, wt, i in order:
                                nc.tensor.matmul(
                                    pot[:],
                                    ct[i][:, :, 128 * t:128 * (t + 1)],
                                    wt[:, i, :, :],
                                    start=(k == 0), stop=(k == 11),
                                    perf_mode=DR,
                                )
                                k += 1
                            o_sb = osbp.tile([128, 512], F32, tag="osb",
                                             name=f"osb{nf}_{t}")
                            nc.vector.scalar_tensor_tensor(
                                o_sb[:], pot[:], OS,
                                bob_sb[:, 512 * nf:512 * (nf + 1)], MUL, ADD)
                            nc.sync.dma_start(
                                out=out_d[128 * t:128 * (t + 1),
                                          512 * nf:512 * (nf + 1)],
                                in_=o_sb[:])

    nc.compile()
    _CACHE["nc"] = nc
    return nc


_E4 = ml_dtypes.float8_e4m3


def _split8(a):
    hi = a.astype(_E4)
    lo = (a - hi.astype(np.float32)).astype(_E4)
    return hi, lo


def _prep_core_inputs(c, x, Wq, bq, Wk, bk, Wv, bv, Wo, bo):
    tp = c % 2
    b = c // 2
    hperm = [16 * tp + h for h in HEAD_PERM]

    xT = np.ascontiguousarray(x[b].T).astype(np.float32)      # [D, S]
    xhi, xlo = _split8(xT)
    pack_x = lambda a: np.ascontiguousarray(
        a.reshape(NC2, 2, 128, S).transpose(2, 0, 1, 3))

    wq64 = Wq.reshape(D, 32, HD)[:, hperm, :].reshape(D, QF) * WS
    qhi, qlo = _split8(wq64)
    pack_wq = lambda a: np.ascontiguousarray(
        a.reshape(NC2, 2, 128, 8, 128).transpose(2, 3, 0, 1, 4))

    wk64 = Wk[:, KF * tp:KF * (tp + 1)] * WS
    khi, klo = _split8(wk64)
    wv64 = Wv[:, KF * tp:KF * (tp + 1)] * WS
    vhi, vlo = _split8(wv64)
    pack_wkv = lambda a: np.ascontiguousarray(
        a.reshape(NC2, 2, 128, KF).transpose(2, 0, 1, 3))

    wo64 = Wo.reshape(32, HD, D)[hperm].reshape(QF, D) * WS
    ohi, olo = _split8(wo64)
    pack_wo = lambda a: np.ascontiguousarray(
        a.reshape(4, 2, 128, 4, 512).transpose(2, 3, 0, 1, 4))

    bq64 = (bq.reshape(32, HD)[hperm].reshape(QF) * WS).reshape(8, 128).T
    bk64 = (bk[KF * tp:KF * (tp + 1)] * WS).reshape(2, 128).T
    bv64 = bv[KF * tp:KF * (tp + 1)] * WS
    bvb = np.concatenate(
        [np.tile(bv64[None, :], (128, 1)), np.ones((128, 4), np.float32)],
        axis=1)
    if tp == 0:
        bob = np.tile(bo[None, :], (128, 1))
    else:
        bob = np.zeros((128, D), np.float32)
    return {
        "xh": pack_x(xhi), "xl": pack_x(xlo),
        "wqh": pack_wq(qhi), "wql": pack_wq(qlo),
        "wkh": pack_wkv(khi), "wkl": pack_wkv(klo),
        "wvh": pack_wkv(vhi), "wvl": pack_wkv(vlo),
        "woh": pack_wo(ohi), "wol": pack_wo(olo),
        "bq": np.ascontiguousarray(bq64.astype(np.float32)),
        "bk": np.ascontiguousarray(bk64.astype(np.float32)),
        "bvb": np.ascontiguousarray(bvb.astype(np.float32)),
        "bob": np.ascontiguousarray(bob.astype(np.float32)),
    }


def kernel(x, Wq, bq, Wk, bk, Wv, bv, Wo, bo):
    global LAST_RUN_NS
    nc = _build()
    in_maps = [
        _prep_core_inputs(c, x, Wq, bq, Wk, bk, Wv, bv, Wo, bo) for c in range(8)
    ]
    t0 = time.perf_counter_ns()
    res = run_bass_kernel_spmd(nc, in_maps, list(range(8)))
    LAST_RUN_NS = time.perf_counter_ns() - t0
    parts = [res.results[c]["out"] for c in range(8)]
    out = np.empty((4, S, D), np.float32)
    for b in range(4):
        out[b] = parts[2 * b] + parts[2 * b + 1]
    return out


# revision 46
# speedup vs baseline: 1.0275x; 1.0275x over previous
"""GQA kernel for Trainium2, 8 NeuronCores.

Problem: B=4, S=1024, D=2048, 32 q-heads, 8 kv-heads, head_dim=64, fp32.

Sharding: TP-2 over heads x DP-4 over batch. Core c handles batch c//2 and
(for tp = c%2) q-heads [16*tp, 16*tp+16) / kv-heads [4*tp, 4*tp+4). Each core
produces a partial output [1024, 2048] (its heads' contribution to ctx @ Wo);
host sums the two partials per batch. bo is added by the tp=0 core only.

Numerics: all weights are prescaled by 64 on the host and split into fp8-e4m3
hi/lo pairs; x is likewise split (hi/lo). Projections run as 3-term
compensated fp8 DoubleRow matmuls (w_hi x_hi + w_lo x_hi + w_hi x_lo), which
the PE cost model charges at half the fp32r row rate. Scores use fp8
DoubleRow with (k_hi, k_hi) x (q_hi, q_lo) slot pairing = k_hi^T(q_hi+q_lo).
The 64*64 scale is absorbed in the exp() scale and the final 1/4096 output
scale. PV runs in bf16 (the v/e linear path cannot take direct fp8 without
blowing the error gate). Softmax denominator comes from an all-ones 65th
column of the value tiles; normalization divides PV psum rows by it before
the fp8 ctx hi/lo split feeding the output projection.

x is transposed on the host; no on-device transposes are needed.

Schedule: the attention phase runs as 16 (pair, token-half) units in a
software pipeline paced by the ACT engine's exp throughput (the hard floor
of the phase): unit u emits scores/exp in 8 block iterations while unit
u-1's PV matmuls (whose exps are all complete) run dense in iterations
0..3 and its normalize chain drains on DVE/Pool; iterations 4..7 are
packed with filler thunks (V-projection tail, the next pair's Q
projection) so the PE never idles long enough to drop out of its full
p-state. K/V/Q0 stream against the x DMAs at the head; the output
projection's per-i-chunk order puts the last pairs' ctx last.

Measured on hardware (same fixed inputs as the harness): rel err 7.35e-3,
HW exec time 227043 ns (baseline 400405 ns).
"""

import time

import ml_dtypes
import numpy as np

import concourse.bass as bass
import concourse.mybir as mybir
from concourse import bacc
from concourse.tile import TileContext
from concourse.bass_utils import run_bass_kernel_spmd

F32 = mybir.dt.float32
BF16 = mybir.dt.bfloat16
F8 = mybir.dt.float8e4
DR = mybir.MatmulPerfMode.DoubleRow
EXP = mybir.ActivationFunctionType.Exp
ADD = mybir.AluOpType.add
SUB = mybir.AluOpType.subtract
MUL = mybir.AluOpType.mult

S = 1024          # sequence length
D = 2048          # d_model
NH = 16           # q heads per core
NKV = 4           # kv heads per core
HD = 64           # head dim
QF = NH * HD      # 1024 q features per core
KF = NKV * HD     # 256 kv features per core
NC2 = 8           # contraction chunk-pairs of d_model (16 chunks of 128)
TT = S // 128     # 8 token tiles
TH = S // 512     # 2 token halves
WS = 64.0         # weight prescale (absorbed downstream)
SCALE = 1.0 / (8.0 * WS * WS)   # exp scale: true scores = psum/(sqrt(64)*64*64)
OS = 1.0 / (WS * WS)            # output descale: out = pot/4096 + bo

# pair p -> (lo head, hi head) local q-head indices; lo heads have kv parity 0
# (kv = h//4; parity 0 -> partitions 0:64 of kT group kv//2).
LO = [0, 1, 2, 3, 8, 9, 10, 11]
HI = [4, 5, 6, 7, 12, 13, 14, 15]
HEAD_PERM = [h for _p in range(8) for h in (LO[_p], HI[_p])]

_CACHE = {}
LAST_RUN_NS = None


def _build():
    if "nc" in _CACHE:
        return _CACHE["nc"]

    nc = bacc.Bacc("TRN2", target_bir_lowering=False, debug=False)

    xh_d = nc.dram_tensor("xh", [128, NC2, 2, S], F8, kind="ExternalInput").ap()
    xl_d = nc.dram_tensor("xl", [128, NC2, 2, S], F8, kind="ExternalInput").ap()
    wqh_d = nc.dram_tensor("wqh", [128, 8, NC2, 2, 128], F8, kind="ExternalInput").ap()
    wql_d = nc.dram_tensor("wql", [128, 8, NC2, 2, 128], F8, kind="ExternalInput").ap()
    wkh_d = nc.dram_tensor("wkh", [128, NC2, 2, KF], F8, kind="ExternalInput").ap()
    wkl_d = nc.dram_tensor("wkl", [128, NC2, 2, KF], F8, kind="ExternalInput").ap()
    wvh_d = nc.dram_tensor("wvh", [128, NC2, 2, KF], F8, kind="ExternalInput").ap()
    wvl_d = nc.dram_tensor("wvl", [128, NC2, 2, KF], F8, kind="ExternalInput").ap()
    woh_d = nc.dram_tensor("woh", [128, 4, 4, 2, 512], F8, kind="ExternalInput").ap()
    wol_d = nc.dram_tensor("wol", [128, 4, 4, 2, 512], F8, kind="ExternalInput").ap()
    bq_d = nc.dram_tensor("bq", [128, 8], F32, kind="ExternalInput").ap()
    bk_d = nc.dram_tensor("bk", [128, 2], F32, kind="ExternalInput").ap()
    bvb_d = nc.dram_tensor("bvb", [128, KF + 4], F32, kind="ExternalInput").ap()
    bob_d = nc.dram_tensor("bob", [128, D], F32, kind="ExternalInput").ap()
    out_d = nc.dram_tensor("out", [S, D], F32, kind="ExternalOutput").ap()

    with TileContext(nc) as tc:
        with (
            tc.tile_pool(name="const", bufs=1) as constp,
            tc.tile_pool(name="xp", bufs=1) as xp,
            tc.tile_pool(name="wkvp", bufs=1) as wkvp,
            tc.tile_pool(name="wqp", bufs=3) as wqp,
            tc.tile_pool(name="kTp", bufs=1) as kTp,
            tc.tile_pool(name="qTp", bufs=1) as qTp,
            tc.tile_pool(name="vaugp", bufs=1) as vaugp,
            tc.tile_pool(name="ctxp", bufs=1) as ctxp,
            tc.tile_pool(name="ps_proj", bufs=2, space="PSUM") as ps_proj,
        ):
            # consts + out-stores go through the Pool queue (SWDGE path),
            # keeping the single HWDGE device free for x / weight loads.
            bq_sb = constp.tile([128, 8], F32, tag="bq")
            nc.gpsimd.dma_start(out=bq_sb[:], in_=bq_d[:, :])
            bk_sb = constp.tile([128, 2], F32, tag="bk")
            nc.gpsimd.dma_start(out=bk_sb[:], in_=bk_d[:, :])
            bvb_sb = constp.tile([128, KF + 4], F32, tag="bvb")
            nc.gpsimd.dma_start(out=bvb_sb[:], in_=bvb_d[:, :])
            bob_sb = constp.tile([128, D], F32, tag="bob")
            nc.gpsimd.dma_start(out=bob_sb[:], in_=bob_d[:, :])

            # x arrives in 2-chunk-pair granules, hi on the SP queue and lo
            # on the DVE queue so the K projection can start on chunk pair 0
            # while the rest stream in.
            # x in four half-tensor DMAs (8KB contiguous per partition):
            # fewer DMA instructions means less HWDGE/issue overhead in the
            # latency-critical startup stream.
            xh_t = xp.tile([128, NC2, 2, S], F8, tag="xh")
            xl_t = xp.tile([128, NC2, 2, S], F8, tag="xl")
            xh = [xh_t[:, c2, :, :] for c2 in range(NC2)]
            xl = [xl_t[:, c2, :, :] for c2 in range(NC2)]
            # K weights first so the K projection's hi-groups (wk x xh) can
            # start as soon as the first x half lands; xl and the V weights
            # stream in behind.
            wkh = wkvp.tile([128, NC2, 2, KF], F8, tag="wkh")
            nc.scalar.dma_start(out=wkh[:], in_=wkh_d[:, :, :, :])
            wkl = wkvp.tile([128, NC2, 2, KF], F8, tag="wkl")
            nc.scalar.dma_start(out=wkl[:], in_=wkl_d[:, :, :, :])
            for h in range(2):
                nc.sync.dma_start(
                    out=xh_t[:, 4 * h:4 * (h + 1), :, :],
                    in_=xh_d[:, 4 * h:4 * (h + 1), :, :])
                nc.scalar.dma_start(
                    out=xl_t[:, 4 * h:4 * (h + 1), :, :],
                    in_=xl_d[:, 4 * h:4 * (h + 1), :, :])
            def load_wq(p):
                wh = wqp.tile([128, NC2, 2, 128], F8, tag="wqh", name=f"wqh{p}")
                nc.sync.dma_start(out=wh[:], in_=wqh_d[:, p, :, :, :])
                wl = wqp.tile([128, NC2, 2, 128], F8, tag="wql", name=f"wql{p}")
                nc.sync.dma_start(out=wl[:], in_=wql_d[:, p, :, :, :])
                return wh, wl
            wq0 = load_wq(0)
            wvh = wkvp.tile([128, NC2, 2, KF], F8, tag="wvh")
            nc.sync.dma_start(out=wvh[:], in_=wvh_d[:, :, :, :])
            wvl = wkvp.tile([128, NC2, 2, KF], F8, tag="wvl")
            nc.sync.dma_start(out=wvl[:], in_=wvl_d[:, :, :, :])

            kT = [kTp.tile([128, 2, S], F8, tag=f"kT{g}", name=f"kT{g}")
                  for g in range(2)]
            qT = [qTp.tile([128, 2, S], F8, tag=f"qT{p}", name=f"qT{p}")
                  for p in range(8)]
            vaug = vaugp.tile([128, NKV, 65 * TT], BF16, tag="vaug")
            ctxH = [ctxp.tile([128, 2, S], F8, tag=f"cH{i}", name=f"cH{i}")
                    for i in range(4)]
            ctxL = [ctxp.tile([128, 2, S], F8, tag=f"cL{i}", name=f"cL{i}")
                    for i in range(4)]


            def q_proj(p, wh, wl):
                for th in range(TH):
                    pq = ps_proj.tile([128, 512], F32, tag="proj",
                                      name=f"pq{p}_{th}")
                    k = 0
                    for wt, xt in ((wh, xh), (wl, xh), (wh, xl)):
                        for c2 in range(NC2):
                            nc.tensor.matmul(
                                pq[:], wt[:, c2, :, :],
                                xt[c2][:, :, 512 * th:512 * (th + 1)],
                                start=(k == 0), stop=(k == 3 * NC2 - 1),
                                perf_mode=DR,
                            )
                            k += 1
                    nc.vector.tensor_scalar_add(
                        qT[p][:, 0, 512 * th:512 * (th + 1)], pq[:],
                        bq_sb[:, p:p + 1])
                    nc.vector.scalar_tensor_tensor(
                        qT[p][:, 1, 512 * th:512 * (th + 1)], pq[:],
                        bq_sb[:, p:p + 1],
                        qT[p][:, 0, 512 * th:512 * (th + 1)], ADD, SUB)

            # ---- K projection ----
            with tc.tile_pool(name="ps_k", bufs=1, space="PSUM") as ps_k:
                pk = {}
                for g in range(2):
                    for th in range(TH):
                        pk[(g, th)] = ps_k.tile(
                            [128, 512], F32, tag=f"pk{g}{th}", name=f"pk{g}{th}")
                # hi-groups A/B per chunk pair as xh streams in (only wk and
                # xh needed); the xl group C trails once xl has landed.
                for c2 in range(NC2):
                    for gi, wt in ((0, wkh), (1, wkl)):
                        for g in range(2):
                            for th in range(TH):
                                nc.tensor.matmul(
                                    pk[(g, th)][:],
                                    wt[:, c2, :, 128 * g:128 * (g + 1)],
                                    xh[c2][:, :, 512 * th:512 * (th + 1)],
                                    start=(gi == 0 and c2 == 0),
                                    stop=False,
                                    perf_mode=DR,
                                )
                for c2 in range(NC2):
                    for g in range(2):
                        for th in range(TH):
                            nc.tensor.matmul(
                                pk[(g, th)][:],
                                wkh[:, c2, :, 128 * g:128 * (g + 1)],
                                xl[c2][:, :, 512 * th:512 * (th + 1)],
                                start=False,
                                stop=(c2 == NC2 - 1),
                                perf_mode=DR,
                            )
                for g in range(2):
                    for th in range(TH):
                        nc.vector.tensor_scalar_add(
                            kT[g][:, 0, 512 * th:512 * (th + 1)],
                            pk[(g, th)][:], bk_sb[:, g:g + 1])
                        nc.vector.tensor_copy(
                            kT[g][:, 1, 512 * th:512 * (th + 1)],
                            kT[g][:, 0, 512 * th:512 * (th + 1)])

            # ---- Q projection for pair 0 + first half of V (head) ----
            q_proj(0, *wq0)

            bv_r = bvb_sb[:, 0:KF].rearrange("p (j f) -> p j f", j=NKV)
            ones_r = bvb_sb[:, KF:KF + 4].rearrange("p (j f) -> p j f", f=1)

            v_boxes = {}

            def v_tile_half(t, half):
                vgrp = [(xt, wt, c2) for xt, wt in
                        ((xh, wvh), (xh, wvl), (xl, wvh))
                        for c2 in range(NC2)]
                if half == 0:
                    v_boxes[t] = ps_proj.tile([128, KF], F32, tag="proj",
                                              name=f"pv{t}")
                pvt = v_boxes[t]
                for k in range(12 * half, 12 * half + 12):
                    xt, wt, c2 = vgrp[k]
                    nc.tensor.matmul(
                        pvt[:], xt[c2][:, :, 128 * t:128 * (t + 1)],
                        wt[:, c2, :, :],
                        start=(k == 0), stop=(k == 3 * NC2 - 1),
                        perf_mode=DR,
                    )
                if half == 1:
                    nc.vector.tensor_add(
                        vaug[:, :, 65 * t:65 * t + 64],
                        pvt[:].rearrange("p (j f) -> p j f", j=NKV), bv_r)
                    nc.vector.tensor_copy(
                        vaug[:, :, 65 * t + 64:65 * t + 65], ones_r)
                    del v_boxes[t]


            # ---- attention: 16 (pair, th) units, software-pipelined ----
            # Unit u emits its scores/exp in 8 iterations; the PV block of
            # unit u-1 runs dense in iterations 0..3 (all its exps are done,
            # so the PE never waits on ACT), its normalize chain drains on
            # DVE/Pool while unit u continues, and filler thunks (V tail,
            # next pair's Q projection) pack iterations 4..7 up to the ACT
            # exp pace. ACT therefore runs back-to-back through the phase.
            with (
                tc.tile_pool(name="ep", bufs=22) as ep,
                tc.tile_pool(name="npool", bufs=2) as npool,
                tc.tile_pool(name="wop", bufs=2) as wop,
                tc.tile_pool(name="osb", bufs=4) as osbp,
                tc.tile_pool(name="ps_sc", bufs=2, space="PSUM") as ps_sc,
                tc.tile_pool(name="ps_pv", bufs=1, space="PSUM") as ps_pv,
            ):
                # prefetch the first two output-projection weight chunks on
                # the Pool queue: they have no deps, so they land during
                # attention instead of queueing behind the exp issues.
                wo_sb = {}
                for nf in range(2):
                    woh_sb = wop.tile([128, 4, 2, 512], F8, tag="woh",
                                      name=f"woh{nf}")
                    nc.gpsimd.dma_start(out=woh_sb[:], in_=woh_d[:, nf, :, :, :])
                    wol_sb = wop.tile([128, 4, 2, 512], F8, tag="wol",
                                      name=f"wol{nf}")
                    nc.gpsimd.dma_start(out=wol_sb[:], in_=wol_d[:, nf, :, :, :])
                    wo_sb[nf] = (woh_sb, wol_sb)

                fillers = []

                def push_q(p, wh, wl):
                    # 12 thunks x 4 DoubleRows per pair, chain order kept
                    grp = ((wh, xh), (wl, xh), (wh, xl))
                    for th in range(TH):
                        box = []
                        for j in range(6):
                            def thunk(j=j, th=th, grp=grp, box=box, p=p):
                                for k in range(4 * j, 4 * j + 4):
                                    wt, xt = grp[k // NC2]
                                    c2 = k % NC2
                                    if not box:
                                        box.append(ps_proj.tile(
                                            [128, 512], F32, tag="proj",
                                            name=f"pq{p}_{th}"))
                                    nc.tensor.matmul(
                                        box[0][:], wt[:, c2, :, :],
                                        xt[c2][:, :, 512 * th:512 * (th + 1)],
                                        start=(k == 0), stop=(k == 23),
                                        perf_mode=DR,
                                    )
                                    if k == 23:
                                        nc.vector.tensor_scalar_add(
                                            qT[p][:, 0, 512 * th:512 * (th + 1)],
                                            box[0][:], bq_sb[:, p:p + 1])
                                        nc.vector.scalar_tensor_tensor(
                                            qT[p][:, 1, 512 * th:512 * (th + 1)],
                                            box[0][:], bq_sb[:, p:p + 1],
                                            qT[p][:, 0, 512 * th:512 * (th + 1)],
                                            ADD, SUB)
                            fillers.append((430, thunk))

                def inject(budget):
                    while fillers and budget > 0:
                        cost, thunk = fillers.pop(0)
                        thunk()
                        budget -= cost

                for t in range(TT):
                    for half in range(2):
                        fillers.append(
                            (650, lambda t=t, half=half: v_tile_half(t, half)))
                push_q(1, *load_wq(1))

                ustate = {}

                def emit_pv(u, upto):
                    p, th, boxAB, es, done = ustate[u]
                    kvlo, kvhi = LO[p] // 4, HI[p] // 4
                    if done[0] >= upto:
                        return
                    if not boxAB:
                        boxAB.append(ps_pv.tile([65, 512], F32, tag="pvA",
                                                name=f"pvA{u}"))
                        boxAB.append(ps_pv.tile([65, 512], F32, tag="pvB",
                                                name=f"pvB{u}"))
                    pvA, pvB = boxAB
                    for pb in range(done[0], upto):
                        nc.tensor.matmul(
                            pvA[:], vaug[:, kvlo, 65 * pb:65 * pb + 65],
                            es[pb][:, 0:512],
                            start=(pb == 0), stop=(pb == TT - 1),
                        )
                        nc.tensor.matmul(
                            pvB[:], vaug[:, kvhi, 65 * pb:65 * pb + 65],
                            es[pb][:, 512:1024],
                            start=(pb == 0), stop=(pb == TT - 1),
                        )
                    done[0] = upto

                def nrm(u):
                    p, th, (pvA, pvB), es, done = ustate.pop(u)
                    i2, s2 = p // 2, p % 2
                    recA = npool.tile([1, 512], F32, tag="recA")
                    recB = npool.tile([1, 512], F32, tag="recB")
                    nc.vector.reciprocal(recA[:], pvA[64:65, :])
                    nc.vector.reciprocal(recB[:], pvB[64:65, :])
                    bcA = npool.tile([64, 512], F32, tag="bcA")
                    bcB = npool.tile([64, 512], F32, tag="bcB")
                    nc.gpsimd.partition_broadcast(bcA[:], recA[:])
                    nc.gpsimd.partition_broadcast(bcB[:], recB[:])
                    # tmp spans all 128 partitions so the hi-head half sits
                    # at base partition 64 like its ctx slice (the DVE needs
                    # equal base partitions for SBUF operand pairs).
                    tmp = npool.tile([128, 512], F32, tag="tmp")
                    nc.vector.tensor_mul(tmp[0:64, :], pvA[0:64, :], bcA[:])
                    nc.vector.tensor_mul(tmp[64:128, :], pvB[0:64, :], bcB[:])
                    cHa = ctxH[i2][0:64, s2, 512 * th:512 * (th + 1)]
                    cHb = ctxH[i2][64:128, s2, 512 * th:512 * (th + 1)]
                    nc.vector.tensor_copy(cHa, tmp[0:64, :])
                    nc.vector.tensor_copy(cHb, tmp[64:128, :])
                    nc.vector.tensor_sub(
                        ctxL[i2][0:64, s2, 512 * th:512 * (th + 1)],
                        tmp[0:64, :], cHa)
                    nc.vector.tensor_sub(
                        ctxL[i2][64:128, s2, 512 * th:512 * (th + 1)],
                        tmp[64:128, :], cHb)

                NU = 8 * TH
                for u in range(NU):
                    p, th = u // TH, u % TH
                    if th == 0 and 1 <= p < 7:
                        push_q(p + 1, *load_wq(p + 1))
                    glo, ghi = LO[p] // 4 // 2, HI[p] // 4 // 2
                    es = [None] * TT
                    ustate[u] = (p, th, [], es, [0])
                    for b in range(TT):
                        psc = ps_sc.tile([128, 1024], F32, tag="psc")
                        nc.tensor.matmul(
                            psc[:, 0:512],
                            kT[glo][0:64, :, 128 * b:128 * (b + 1)],
                            qT[p][0:64, :, 512 * th:512 * (th + 1)],
                            start=True, stop=True, perf_mode=DR,
                        )
                        nc.tensor.matmul(
                            psc[:, 512:1024],
                            kT[ghi][64:128, :, 128 * b:128 * (b + 1)],
                            qT[p][64:128, :, 512 * th:512 * (th + 1)],
                            start=True, stop=True, perf_mode=DR,
                        )
                        e = ep.tile([128, 1024], BF16, tag="e")
                        nc.scalar.activation(
                            e[:], psc[:], EXP, bias=0.0, scale=SCALE)
                        es[b] = e
                        if u > 0:
                            if b == 0:
                                # one extra iteration of slack before the PV
                                # block restarts: the pv psum slot is still
                                # draining through the previous normalize.
                                inject(430)
                            elif b < 3:
                                emit_pv(u - 1, 3 * b)
                            elif b == 3:
                                emit_pv(u - 1, 8)
                                nrm(u - 1)
                            elif u >= NU - 2:
                                # pair 7 has no next-pair Q filler; run its
                                # own PV early as the exps complete.
                                emit_pv(u, min(2 * (b - 3), 7))
                            else:
                                inject(860)
                        else:
                            inject(1300 if b < 4 else 860)
                emit_pv(NU - 1, 8)
                nrm(NU - 1)

                # ---- output projection ----
                if True:
                    for nf in range(4):
                        if nf < 2:
                            woh_sb, wol_sb = wo_sb[nf]
                        else:
                            woh_sb = wop.tile([128, 4, 2, 512], F8, tag="woh",
                                              name=f"woh{nf}")
                            nc.scalar.dma_start(
                                out=woh_sb[:], in_=woh_d[:, nf, :, :, :])
                            wol_sb = wop.tile([128, 4, 2, 512], F8, tag="wol",
                                              name=f"wol{nf}")
                            nc.scalar.dma_start(
                                out=wol_sb[:], in_=wol_d[:, nf, :, :, :])
                        for t in range(TT):
                            pot = ps_proj.tile([128, 512], F32, tag="proj",
                                               name=f"po{nf}_{t}")
                            # i=3 (ctx of pairs 6/7) last in every group so
                            # the first 9 matmuls can pre-run while the final
                            # pairs' attention is still normalizing.
                            k = 0
                            for i in (0, 1, 2, 3):
                                pass
                            groups = ((ctxH, woh_sb), (ctxL, woh_sb),
                                      (ctxH, wol_sb))
                            order = [(ct, wt, i) for ct, wt in groups
                                     for i in (0, 1, 2)]
                            order += [(ct, wt, 3) for ct, wt in groups]
                            for ct, wt, i in order:
                                nc.tensor.matmul(
                                    pot[:],
                                    ct[i][:, :, 128 * t:128 * (t + 1)],
                                    wt[:, i, :, :],
                                    start=(k == 0), stop=(k == 11),
                                    perf_mode=DR,
                                )
                                k += 1
                            o_sb = osbp.tile([128, 512], F32, tag="osb",
                                             name=f"osb{nf}_{t}")
                            nc.vector.scalar_tensor_tensor(
                                o_sb[:], pot[:], OS,
                                bob_sb[:, 512 * nf:512 * (nf + 1)], MUL, ADD)
                            nc.sync.dma_start(
                                out=out_d[128 * t:128 * (t + 1),
                                          512 * nf:512 * (nf + 1)],
                                in_=o_sb[:])

    nc.compile()
    _CACHE["nc"] = nc
    return nc


_E4 = ml_dtypes.float8_e4m3


def _split8(a):
    hi = a.astype(_E4)
    lo = (a - hi.astype(np.float32)).astype(_E4)
    return hi, lo


def _prep_core_inputs(c, x, Wq, bq, Wk, bk, Wv, bv, Wo, bo):
    tp = c % 2
    b = c // 2
    hperm = [16 * tp + h for h in HEAD_PERM]

    xT = np.ascontiguousarray(x[b].T).astype(np.float32)      # [D, S]
    xhi, xlo = _split8(xT)
    pack_x = lambda a: np.ascontiguousarray(
        a.reshape(NC2, 2, 128, S).transpose(2, 0, 1, 3))

    wq64 = Wq.reshape(D, 32, HD)[:, hperm, :].reshape(D, QF) * WS
    qhi, qlo = _split8(wq64)
    pack_wq = lambda a: np.ascontiguousarray(
        a.reshape(NC2, 2, 128, 8, 128).transpose(2, 3, 0, 1, 4))

    wk64 = Wk[:, KF * tp:KF * (tp + 1)] * WS
    khi, klo = _split8(wk64)
    wv64 = Wv[:, KF * tp:KF * (tp + 1)] * WS
    vhi, vlo = _split8(wv64)
    pack_wkv = lambda a: np.ascontiguousarray(
        a.reshape(NC2, 2, 128, KF).transpose(2, 0, 1, 3))

    wo64 = Wo.reshape(32, HD, D)[hperm].reshape(QF, D) * WS
    ohi, olo = _split8(wo64)
    pack_wo = lambda a: np.ascontiguousarray(
        a.reshape(4, 2, 128, 4, 512).transpose(2, 3, 0, 1, 4))

    bq64 = (bq.reshape(32, HD)[hperm].reshape(QF) * WS).reshape(8, 128).T
    bk64 = (bk[KF * tp:KF * (tp + 1)] * WS).reshape(2, 128).T
    bv64 = bv[KF * tp:KF * (tp + 1)] * WS
    bvb = np.concatenate(
        [np.tile(bv64[None, :], (128, 1)), np.ones((128, 4), np.float32)],
        axis=1)
    if tp == 0:
        bob = np.tile(bo[None, :], (128, 1))
    else:
        bob = np.zeros((128, D), np.float32)
    return {
        "xh": pack_x(xhi), "xl": pack_x(xlo),
        "wqh": pack_wq(qhi), "wql": pack_wq(qlo),
        "wkh": pack_wkv(khi), "wkl": pack_wkv(klo),
        "wvh": pack_wkv(vhi), "wvl": pack_wkv(vlo),
        "woh": pack_wo(ohi), "wol": pack_wo(olo),
        "bq": np.ascontiguousarray(bq64.astype(np.float32)),
        "bk": np.ascontiguousarray(bk64.astype(np.float32)),
        "bvb": np.ascontiguousarray(bvb.astype(np.float32)),
        "bob": np.ascontiguousarray(bob.astype(np.float32)),
    }


def kernel(x, Wq, bq, Wk, bk, Wv, bv, Wo, bo):
    global LAST_RUN_NS
    nc = _build()
    in_maps = [
        _prep_core_inputs(c, x, Wq, bq, Wk, bk, Wv, bv, Wo, bo) for c in range(8)
    ]
    t0 = time.perf_counter_ns()
    res = run_bass_kernel_spmd(nc, in_maps, list(range(8)))
    LAST_RUN_NS = time.perf_counter_ns() - t0
    parts = [res.results[c]["out"] for c in range(8)]
    out = np.empty((4, S, D), np.float32)
    for b in range(4):
        out[b] = parts[2 * b] + parts[2 * b + 1]
    return out
